# revision 24
# baseline (speedup 1.0000x reference)
"""Trainium2 Bass kernel for CustomConv2d:
  x [16, 32, 512, 512] f32, weight [32, 32, 3, 3] f32, bias [32] f32
  -> out [16, 32, 510, 510] f32   (stride 1, VALID padding, + bias)

Data-parallel over batch: 2 images per core across 8 NeuronCores.

v9 design (v8 + fine-grained paced input chunks):
 - Host converts x/weight to bf16 and relayouts x into the exact SBUF strip
   layout. Input is DMAed in 512KB chunks of [128, 2048] (4KB descriptors,
   8 output rows each), issued ~10 chunks ahead of the consuming block, so
   the DMA engines' packet-granular round-robin between the load and store
   queues stays fair. (v8's 4MB pair loads made ~262KB/engine packets that
   starved the store queue 9:1, backing up ob recycle -> drains -> PSUM ->
   15us + periodic ~4us PE stalls; PE is the real floor at ~182us.) Output
   accumulates four blocks per [128, 2040] bf16 tile (4080B descriptors)
   and is unshuffled + upcast to f32 on the host.
 - SBUF x layout: strip s holds input rows 32s..32s+31; partition
   64*img + 32*(r%2) + ci, free offset 512*((r%32)//2) + w. One [128, 8192]
   bf16 tile per strip covers both images of the core.
 - Compute: per output row pair (y, y+1), y even: one 6-MM accumulation
   chain of [K=64, M=64] matmuls -- 3 kw taps x 2 two-row input windows,
   with 3-of-4 weight blocks useful per MM (75%, the ceiling for 2-row
   windows). Chains are tile-pure and banks row-half-pure: HW crashes if an
   accumulation group spans PE row tiles or a PSUM bank is written by more
   than one row quadrant (verified by probing), and the PE sustains only ~8
   concurrent matmul streams, so 4 tiles of [64,64] keep the whole array
   busy with big, cheap-to-issue matmuls (3072 total; LDWEIGHTS has ~90ns
   fixed cost and ~2.6x concurrency, so 18k+ small matmuls are issue-bound).
 - Blocks of 4 output rows (2 pairs): pair cp -> PSUM bank (il) partitions
   64cp+32h+co = row y0+2cp+h. 2 banks per block, bufs=4 -> all 8 banks.
 - Drains are full [128, 510] bias-adds psum->bf16, alternating ScalarE
   (img0) / VectorE (img1). Row block 127 is y0=506 (recomputes rows
   506/507) so all drains stay uniform; host takes rows 508/509 from it.
"""
import numpy as np
from ml_dtypes import bfloat16

import concourse.bass as bass
import concourse.tile as tile
from concourse import bacc, mybir
from concourse.bass_utils import run_bass_kernel_spmd
from contextlib import ExitStack

F32 = mybir.dt.float32
BF16 = mybir.dt.bfloat16

N_FULL, C, H, W = 16, 32, 512, 512
HO = WO = 510
N_CORES = 8
N_PER = N_FULL // N_CORES          # 2 images per core
N_STRIPS = H // 32                 # 16 strips of 32 input rows
N_BLOCKS = 128                     # 4-output-row blocks (block 127: y0=506)


def _block_y0(mb):
    return 4 * mb if mb < N_BLOCKS - 1 else 506


CHUNK_COLS = 4096                  # per-chunk free dim: 8 t-groups = 16 rows
N_CHUNKS = 32                      # 512 rows / 16 rows per chunk
CHUNK_AHEAD = 11                   # chunks issued ahead of the consuming block
X_BUFS = 13                        # chunk ring depth (13 MB SBUF)
OB_BLOCKS = 8                      # output blocks batched per [128, 8*WO] tile


def _build():
    nc = bacc.Bacc("TRN2", target_bir_lowering=False, debug=False, num_devices=1)
    x_d = nc.dram_tensor("x", [N_STRIPS // 2, 128, 16384], BF16,
                         kind="ExternalInput").ap()
    w_d = nc.dram_tensor("w", [128, 384], BF16, kind="ExternalInput").ap()
    b_d = nc.dram_tensor("b", [128, 1], F32, kind="ExternalInput").ap()
    # rows 0..15: il0 groups (blocks 8p..8p+7); rows 16..32: il1 groups
    # staggered by 4 blocks (g covers blocks 8g-4..8g+3), so 1MB stores fire
    # every 4 blocks instead of 2MB bursts every 8.
    o_d = nc.dram_tensor("out", [33, 128, OB_BLOCKS * WO],
                         BF16, kind="ExternalOutput").ap()

    with tile.TileContext(nc) as tc, ExitStack() as ctx:
        const_pool = ctx.enter_context(tc.tile_pool(name="const", bufs=1))
        x_pool = ctx.enter_context(tc.tile_pool(name="xs", bufs=X_BUFS))
        ps_pool = ctx.enter_context(tc.tile_pool(name="ps", bufs=4, space="PSUM"))
        o_pool = ctx.enter_context(tc.tile_pool(name="ob", bufs=3))

        wv = const_pool.tile([128, 384], BF16)
        nc.sync.dma_start(wv[:], w_d[:])
        bt = const_pool.tile([128, 1], F32)
        nc.sync.dma_start(bt[:], b_d[:])

        xtiles = {}
        obtiles = {}

        # Input is DMAed in 512 KB chunks (4 KB descriptors) so the DMA
        # engines' packet-granular round-robin between the load queue and the
        # store queue stays fair; 4 MB strip-pair loads monopolized the
        # engines for ~10 us per packet, starving stores -> ob recycle ->
        # drains -> PSUM -> PE stalls (15 us + ~4 us per strip boundary).
        def load_chunk(c):
            # Loads issue from sync (SP ring), stores from scalar (ACT ring):
            # separate rings so the engines round-robin the two queues at
            # matched ~2.3us packet granularity (one shared FIFO ring was
            # tried: 216us vs 207us, head-of-line blocking). Loads must NOT
            # issue from scalar: the ~0.6us dma_start sequencer cost lands
            # between bias-drains and delays PSUM recycle -> ~1us PE stalls.
            xa = x_pool.tile([128, CHUNK_COLS], BF16, tag="x", name=f"xc_{c}")
            s2, lc = divmod(c, 16384 // CHUNK_COLS)
            nc.scalar.dma_start(
                xa[:], x_d[s2][:, CHUNK_COLS * lc:CHUNK_COLS * (lc + 1)])
            xtiles[2 * c] = (xa, 0)
            xtiles[2 * c + 1] = (xa, CHUNK_COLS // 2)

        def emit_block(mb):
            y0 = _block_y0(mb)
            banks = {}
            for il in range(N_PER):
                banks[il] = ps_pool.tile([128, 512], F32, tag=f"ps{il}",
                                         name=f"ps{il}_{mb}")
            for step in range(6):
                w, kw = divmod(step, 3)
                for il in range(N_PER):
                    for cp in range(2):
                        rw = y0 + 2 * cp + 2 * w       # window rows rw, rw+1
                        xa, base = xtiles[rw // 8]
                        off = base + 512 * ((rw % 8) // 2) + kw
                        nc.tensor.matmul(
                            banks[il][64 * cp:64 * cp + 64, 0:WO],
                            wv[64 * il:64 * il + 64,
                               64 * (3 * w + kw):64 * (3 * w + kw) + 64],
                            xa[64 * il:64 * il + 64, off:off + WO],
                            start=(step == 0), stop=(step == 5),
                            skip_group_check=True,
                            tile_position=(64 * il, 64 * cp),
                        )
            # OB_BLOCKS consecutive blocks share one [128, OB_BLOCKS*510]
            # output tile per image -> 1MB stores whose per-engine packet
            # duration matches the 1MB input chunks (fair round-robin).
            # il1 groups are staggered +4 blocks; first/last groups store in
            # 0.5MB halves so the post-last-matmul tail is short.
            for il in range(N_PER):
                g, half = divmod(mb + 4 * il, OB_BLOCKS)
                if half == 0 or mb == 0:
                    obtiles[il] = o_pool.tile([128, OB_BLOCKS * WO], BF16,
                                              tag=f"ob{il}", name=f"ob{il}_{g}")
                ob = obtiles[il]
                src = banks[il][0:128, 0:WO]
                dst = ob[:, half * WO:half * WO + WO]
                if il == 0:
                    nc.scalar.activation(
                        dst, src, mybir.ActivationFunctionType.Identity,
                        bias=bt[:])
                else:
                    nc.vector.tensor_scalar_add(dst, src, bt[:])
                row = 16 * il + g
                hw = OB_BLOCKS * WO // 2
                last_g = 15 + il                 # 15 for il0, 16 for il1
                first_half_g = (half == OB_BLOCKS // 2 - 1 and g == last_g)
                if half == OB_BLOCKS - 1:
                    if (il == 1 and g == 0) or g == last_g:
                        nc.sync.dma_start(o_d[row][:, hw:], ob[:, hw:])
                    else:
                        nc.sync.dma_start(o_d[row], ob[:])
                elif first_half_g:
                    nc.sync.dma_start(o_d[row][:, 0:hw], ob[:, 0:hw])

        # chunk 0 loads as two 0.5MB halves so the first matmul's input
        # dependency is half as deep (head trim).
        for i in range(2):
            xh = x_pool.tile([128, CHUNK_COLS // 2], BF16, tag="x0", bufs=2,
                             name=f"xc0_{i}")
            nc.scalar.dma_start(
                xh[:], x_d[0][:, (CHUNK_COLS // 2) * i:(CHUNK_COLS // 2) * (i + 1)])
            xtiles[i] = (xh, 0)

        next_chunk = 1
        for mb in range(N_BLOCKS):
            y0 = _block_y0(mb)
            target = min((y0 + 5) // 16 + CHUNK_AHEAD, N_CHUNKS - 1)
            while next_chunk <= target:
                load_chunk(next_chunk)
                next_chunk += 1
            emit_block(mb)

    nc.compile()
    return nc


def _prep_inputs(x, weight, bias):
    """Host-side shard + relayout. Returns per-core in_maps."""
    x = np.asarray(x, dtype=np.float32)
    weight = np.asarray(weight, dtype=np.float32)
    bias = np.asarray(bias, dtype=np.float32)

    # x[2i+il, ci, 32(2*s2+sodd)+2t+q, w]
    #   -> xs[i, s2, 64*il+32*q+ci, 8192*sodd + 512*t + w]
    xr = x.reshape(N_CORES, N_PER, C, N_STRIPS // 2, 2, 16, 2, W)
    xr = xr.transpose(0, 3, 1, 6, 2, 4, 5, 7)   # core, s2, il, q, ci, sodd, t, w
    xs = np.ascontiguousarray(xr).reshape(N_CORES, N_STRIPS // 2, 128, 16384)
    xs = xs.astype(bfloat16)

    # wv[64il + 32q + ci, 64*(3w+kw) + 32h + co] = weight[co, ci, 2w+q-h, kw]
    # (zero when kh = 2w+q-h is outside [0, 3))
    wk = np.zeros((2, 32, 6, 2, 32), dtype=np.float32)  # q, ci, (w,kw), h, co
    for w in range(2):
        for kw in range(3):
            for q in range(2):
                for h in range(2):
                    kh = 2 * w + q - h
                    if 0 <= kh <= 2:
                        wk[q, :, 3 * w + kw, h, :] = weight[:, :, kh, kw].T
    wv = wk.transpose(0, 1, 2, 3, 4).reshape(64, 384)
    wv = np.tile(wv, (2, 1)).astype(bfloat16)
    bt = np.tile(bias, 4)[:, None].astype(np.float32)

    return [{"x": xs[i], "w": wv, "b": bt} for i in range(N_CORES)]


def _unpack_output(results):
    """results: list of 8 dicts with 'out' [33, 128, 4080] bf16.
    Rows 0..15: il0 groups; rows 16..32: il1 groups staggered +4 blocks."""
    dev = np.stack([r["out"] for r in results], axis=0)

    def asm(part):
        # part [cores, G, 128 part = 64cp+32h+co, 8half*510] ->
        # [cores, C, 32G composed rows = 32g+4half+2cp+h, 510]
        G = part.shape[1]
        p = part.reshape(N_CORES, G, 2, 2, C, OB_BLOCKS, WO)
        p = p.transpose(0, 4, 1, 5, 2, 3, 6)
        return p.reshape(N_CORES, C, 32 * G, WO)

    il0 = asm(dev[:, 0:16])                  # composed row = 4mb + 2cp + h
    il1 = asm(dev[:, 16:33])[:, :, 16:528]   # shift out the -4-block stagger
    full = np.stack([il0, il1], axis=1)      # [cores, il, C, 512, WO]
    out = np.empty((N_FULL, C, HO, WO), dtype=np.float32)
    o = out.reshape(N_CORES, N_PER, C, HO, WO)
    o[:, :, :, :508, :] = full[:, :, :, :508, :]
    # block 127 recomputes at y0=506: composed rows 510/511 = true 508/509
    o[:, :, :, 508:510, :] = full[:, :, :, 510:512, :]
    return out


_NC = None


def kernel(x, weight, bias):
    global _NC
    if _NC is None:
        _NC = _build()
    in_maps = _prep_inputs(x, weight, bias)
    res = run_bass_kernel_spmd(_NC, in_maps, core_ids=list(range(N_CORES)))
    return _unpack_output(res.results)



# revision 26
# speedup vs baseline: 1.0146x; 1.0146x over previous
"""Trainium2 Bass kernel for CustomConv2d:
  x [16, 32, 512, 512] f32, weight [32, 32, 3, 3] f32, bias [32] f32
  -> out [16, 32, 510, 510] f32   (stride 1, VALID padding, + bias)

Data-parallel over batch: 2 images per core across 8 NeuronCores.

v9 design (v8 + fine-grained paced input chunks):
 - Host converts x/weight to bf16 and relayouts x into the exact SBUF strip
   layout. Input is DMAed in 512KB chunks of [128, 2048] (4KB descriptors,
   8 output rows each), issued ~10 chunks ahead of the consuming block, so
   the DMA engines' packet-granular round-robin between the load and store
   queues stays fair. (v8's 4MB pair loads made ~262KB/engine packets that
   starved the store queue 9:1, backing up ob recycle -> drains -> PSUM ->
   15us + periodic ~4us PE stalls; PE is the real floor at ~182us.) Output
   accumulates four blocks per [128, 2040] bf16 tile (4080B descriptors)
   and is unshuffled + upcast to f32 on the host.
 - SBUF x layout: strip s holds input rows 32s..32s+31; partition
   64*img + 32*(r%2) + ci, free offset 512*((r%32)//2) + w. One [128, 8192]
   bf16 tile per strip covers both images of the core.
 - Compute: per output row pair (y, y+1), y even: one 6-MM accumulation
   chain of [K=64, M=64] matmuls -- 3 kw taps x 2 two-row input windows,
   with 3-of-4 weight blocks useful per MM (75%, the ceiling for 2-row
   windows). Chains are tile-pure and banks row-half-pure: HW crashes if an
   accumulation group spans PE row tiles or a PSUM bank is written by more
   than one row quadrant (verified by probing), and the PE sustains only ~8
   concurrent matmul streams, so 4 tiles of [64,64] keep the whole array
   busy with big, cheap-to-issue matmuls (3072 total; LDWEIGHTS has ~90ns
   fixed cost and ~2.6x concurrency, so 18k+ small matmuls are issue-bound).
 - Blocks of 4 output rows (2 pairs): pair cp -> PSUM bank (il) partitions
   64cp+32h+co = row y0+2cp+h. 2 banks per block, bufs=4 -> all 8 banks.
 - Drains are full [128, 510] bias-adds psum->bf16, alternating ScalarE
   (img0) / VectorE (img1). Row block 127 is y0=506 (recomputes rows
   506/507) so all drains stay uniform; host takes rows 508/509 from it.
"""
import numpy as np
from ml_dtypes import bfloat16

import concourse.bass as bass
import concourse.tile as tile
from concourse import bacc, mybir
from concourse.bass_utils import run_bass_kernel_spmd
from contextlib import ExitStack

F32 = mybir.dt.float32
BF16 = mybir.dt.bfloat16

N_FULL, C, H, W = 16, 32, 512, 512
HO = WO = 510
N_CORES = 8
N_PER = N_FULL // N_CORES          # 2 images per core
N_STRIPS = H // 32                 # 16 strips of 32 input rows
N_BLOCKS = 128                     # 4-output-row blocks (block 127: y0=506)


def _block_y0(mb):
    return 4 * mb if mb < N_BLOCKS - 1 else 506


CHUNK_COLS = 4096                  # per-chunk free dim: 8 t-groups = 16 rows
N_CHUNKS = 32                      # 512 rows / 16 rows per chunk
CHUNK_AHEAD = 6                    # chunks issued ahead of the consuming block
X_BUFS = 8                         # chunk ring depth (8 MB SBUF)
OB_BLOCKS = 8                      # output blocks batched per [128, 8*WO] tile


def _build():
    nc = bacc.Bacc("TRN2", target_bir_lowering=False, debug=False, num_devices=1)
    x_d = nc.dram_tensor("x", [N_STRIPS // 2, 128, 16384], BF16,
                         kind="ExternalInput").ap()
    w_d = nc.dram_tensor("w", [128, 384], BF16, kind="ExternalInput").ap()
    b_d = nc.dram_tensor("b", [128, 1], F32, kind="ExternalInput").ap()
    # rows 0..15: il0 groups (blocks 8p..8p+7); rows 16..32: il1 groups
    # staggered by 4 blocks (g covers blocks 8g-4..8g+3), so 1MB stores fire
    # every 4 blocks instead of 2MB bursts every 8.
    o_d = nc.dram_tensor("out", [33, 128, OB_BLOCKS * WO],
                         BF16, kind="ExternalOutput").ap()

    with tile.TileContext(nc) as tc, ExitStack() as ctx:
        const_pool = ctx.enter_context(tc.tile_pool(name="const", bufs=1))
        x_pool = ctx.enter_context(tc.tile_pool(name="xs", bufs=X_BUFS))
        ps_pool = ctx.enter_context(tc.tile_pool(name="ps", bufs=4, space="PSUM"))
        o_pool = ctx.enter_context(tc.tile_pool(name="ob", bufs=4))

        wv = const_pool.tile([128, 384], BF16)
        nc.sync.dma_start(wv[:], w_d[:])
        bt = const_pool.tile([128, 1], F32)
        nc.sync.dma_start(bt[:], b_d[:])

        xtiles = {}
        obtiles = {}

        # Input is DMAed in 512 KB chunks (4 KB descriptors) so the DMA
        # engines' packet-granular round-robin between the load queue and the
        # store queue stays fair; 4 MB strip-pair loads monopolized the
        # engines for ~10 us per packet, starving stores -> ob recycle ->
        # drains -> PSUM -> PE stalls (15 us + ~4 us per strip boundary).
        def load_chunk(c):
            # Loads issue from sync (SP ring), stores from scalar (ACT ring):
            # separate rings so the engines round-robin the two queues at
            # matched ~2.3us packet granularity (one shared FIFO ring was
            # tried: 216us vs 207us, head-of-line blocking). Loads must NOT
            # issue from scalar: the ~0.6us dma_start sequencer cost lands
            # between bias-drains and delays PSUM recycle -> ~1us PE stalls.
            xa = x_pool.tile([128, CHUNK_COLS], BF16, tag="x", name=f"xc_{c}")
            s2, lc = divmod(c, 16384 // CHUNK_COLS)
            nc.scalar.dma_start(
                xa[:], x_d[s2][:, CHUNK_COLS * lc:CHUNK_COLS * (lc + 1)])
            xtiles[2 * c] = (xa, 0)
            xtiles[2 * c + 1] = (xa, CHUNK_COLS // 2)

        def emit_block(mb):
            y0 = _block_y0(mb)
            banks = {}
            for il in range(N_PER):
                banks[il] = ps_pool.tile([128, 512], F32, tag=f"ps{il}",
                                         name=f"ps{il}_{mb}")
            for step in range(6):
                w, kw = divmod(step, 3)
                for il in range(N_PER):
                    for cp in range(2):
                        rw = y0 + 2 * cp + 2 * w       # window rows rw, rw+1
                        xa, base = xtiles[rw // 8]
                        off = base + 512 * ((rw % 8) // 2) + kw
                        nc.tensor.matmul(
                            banks[il][64 * cp:64 * cp + 64, 0:WO],
                            wv[64 * il:64 * il + 64,
                               64 * (3 * w + kw):64 * (3 * w + kw) + 64],
                            xa[64 * il:64 * il + 64, off:off + WO],
                            start=(step == 0), stop=(step == 5),
                            skip_group_check=True,
                            tile_position=(64 * il, 64 * cp),
                        )
            # OB_BLOCKS consecutive blocks share one [128, OB_BLOCKS*510]
            # output tile per image -> 1MB stores whose per-engine packet
            # duration matches the 1MB input chunks (fair round-robin).
            # il1 groups are staggered +4 blocks; first/last groups store in
            # 0.5MB halves so the post-last-matmul tail is short.
            for il in range(N_PER):
                g, half = divmod(mb + 4 * il, OB_BLOCKS)
                if half == 0 or mb == 0:
                    obtiles[il] = o_pool.tile([128, OB_BLOCKS * WO], BF16,
                                              tag=f"ob{il}", name=f"ob{il}_{g}")
                ob = obtiles[il]
                src = banks[il][0:128, 0:WO]
                dst = ob[:, half * WO:half * WO + WO]
                if il == 0:
                    nc.scalar.activation(
                        dst, src, mybir.ActivationFunctionType.Identity,
                        bias=bt[:])
                else:
                    nc.vector.tensor_scalar_add(dst, src, bt[:])
                row = 16 * il + g
                hw = OB_BLOCKS * WO // 2
                last_g = 15 + il                 # 15 for il0, 16 for il1
                first_half_g = (half == OB_BLOCKS // 2 - 1 and g == last_g)
                if half == OB_BLOCKS - 1:
                    if (il == 1 and g == 0) or g == last_g:
                        nc.sync.dma_start(o_d[row][:, hw:], ob[:, hw:])
                    else:
                        nc.sync.dma_start(o_d[row], ob[:])
                elif first_half_g:
                    nc.sync.dma_start(o_d[row][:, 0:hw], ob[:, 0:hw])

        # chunk 0 loads as two 0.5MB halves so the first matmul's input
        # dependency is half as deep (head trim).
        for i in range(2):
            xh = x_pool.tile([128, CHUNK_COLS // 2], BF16, tag="x0", bufs=2,
                             name=f"xc0_{i}")
            nc.scalar.dma_start(
                xh[:], x_d[0][:, (CHUNK_COLS // 2) * i:(CHUNK_COLS // 2) * (i + 1)])
            xtiles[i] = (xh, 0)

        next_chunk = 1
        for mb in range(N_BLOCKS):
            y0 = _block_y0(mb)
            target = min((y0 + 5) // 16 + CHUNK_AHEAD, N_CHUNKS - 1)
            while next_chunk <= target:
                load_chunk(next_chunk)
                next_chunk += 1
            emit_block(mb)

    nc.compile()
    return nc


def _prep_inputs(x, weight, bias):
    """Host-side shard + relayout. Returns per-core in_maps."""
    x = np.asarray(x, dtype=np.float32)
    weight = np.asarray(weight, dtype=np.float32)
    bias = np.asarray(bias, dtype=np.float32)

    # x[2i+il, ci, 32(2*s2+sodd)+2t+q, w]
    #   -> xs[i, s2, 64*il+32*q+ci, 8192*sodd + 512*t + w]
    xr = x.reshape(N_CORES, N_PER, C, N_STRIPS // 2, 2, 16, 2, W)
    xr = xr.transpose(0, 3, 1, 6, 2, 4, 5, 7)   # core, s2, il, q, ci, sodd, t, w
    xs = np.ascontiguousarray(xr).reshape(N_CORES, N_STRIPS // 2, 128, 16384)
    xs = xs.astype(bfloat16)

    # wv[64il + 32q + ci, 64*(3w+kw) + 32h + co] = weight[co, ci, 2w+q-h, kw]
    # (zero when kh = 2w+q-h is outside [0, 3))
    wk = np.zeros((2, 32, 6, 2, 32), dtype=np.float32)  # q, ci, (w,kw), h, co
    for w in range(2):
        for kw in range(3):
            for q in range(2):
                for h in range(2):
                    kh = 2 * w + q - h
                    if 0 <= kh <= 2:
                        wk[q, :, 3 * w + kw, h, :] = weight[:, :, kh, kw].T
    wv = wk.transpose(0, 1, 2, 3, 4).reshape(64, 384)
    wv = np.tile(wv, (2, 1)).astype(bfloat16)
    bt = np.tile(bias, 4)[:, None].astype(np.float32)

    return [{"x": xs[i], "w": wv, "b": bt} for i in range(N_CORES)]


def _unpack_output(results):
    """results: list of 8 dicts with 'out' [33, 128, 4080] bf16.
    Rows 0..15: il0 groups; rows 16..32: il1 groups staggered +4 blocks."""
    dev = np.stack([r["out"] for r in results], axis=0)

    def asm(part):
        # part [cores, G, 128 part = 64cp+32h+co, 8half*510] ->
        # [cores, C, 32G composed rows = 32g+4half+2cp+h, 510]
        G = part.shape[1]
        p = part.reshape(N_CORES, G, 2, 2, C, OB_BLOCKS, WO)
        p = p.transpose(0, 4, 1, 5, 2, 3, 6)
        return p.reshape(N_CORES, C, 32 * G, WO)

    il0 = asm(dev[:, 0:16])                  # composed row = 4mb + 2cp + h
    il1 = asm(dev[:, 16:33])[:, :, 16:528]   # shift out the -4-block stagger
    full = np.stack([il0, il1], axis=1)      # [cores, il, C, 512, WO]
    out = np.empty((N_FULL, C, HO, WO), dtype=np.float32)
    o = out.reshape(N_CORES, N_PER, C, HO, WO)
    o[:, :, :, :508, :] = full[:, :, :, :508, :]
    # block 127 recomputes at y0=506: composed rows 510/511 = true 508/509
    o[:, :, :, 508:510, :] = full[:, :, :, 510:512, :]
    return out


_NC = None


def kernel(x, weight, bias):
    global _NC
    if _NC is None:
        _NC = _build()
    in_maps = _prep_inputs(x, weight, bias)
    res = run_bass_kernel_spmd(_NC, in_maps, core_ids=list(range(N_CORES)))
    return _unpack_output(res.results)



# revision 27
# speedup vs baseline: 1.0892x; 1.0735x over previous
"""Trainium2 Bass kernel for CustomConv2d:
  x [16, 32, 512, 512] f32, weight [32, 32, 3, 3] f32, bias [32] f32
  -> out [16, 32, 510, 510] f32   (stride 1, VALID padding, + bias)

Data-parallel over batch: 2 images per core across 8 NeuronCores.

v9 design (v8 + fine-grained paced input chunks):
 - Host converts x/weight to bf16 and relayouts x into the exact SBUF strip
   layout. Input is DMAed in 512KB chunks of [128, 2048] (4KB descriptors,
   8 output rows each), issued ~10 chunks ahead of the consuming block, so
   the DMA engines' packet-granular round-robin between the load and store
   queues stays fair. (v8's 4MB pair loads made ~262KB/engine packets that
   starved the store queue 9:1, backing up ob recycle -> drains -> PSUM ->
   15us + periodic ~4us PE stalls; PE is the real floor at ~182us.) Output
   accumulates four blocks per [128, 2040] bf16 tile (4080B descriptors)
   and is unshuffled + upcast to f32 on the host.
 - SBUF x layout: strip s holds input rows 32s..32s+31; partition
   64*img + 32*(r%2) + ci, free offset 512*((r%32)//2) + w. One [128, 8192]
   bf16 tile per strip covers both images of the core.
 - Compute: per output row pair (y, y+1), y even: one 6-MM accumulation
   chain of [K=64, M=64] matmuls -- 3 kw taps x 2 two-row input windows,
   with 3-of-4 weight blocks useful per MM (75%, the ceiling for 2-row
   windows). Chains are tile-pure and banks row-half-pure: HW crashes if an
   accumulation group spans PE row tiles or a PSUM bank is written by more
   than one row quadrant (verified by probing), and the PE sustains only ~8
   concurrent matmul streams, so 4 tiles of [64,64] keep the whole array
   busy with big, cheap-to-issue matmuls (3072 total; LDWEIGHTS has ~90ns
   fixed cost and ~2.6x concurrency, so 18k+ small matmuls are issue-bound).
 - Blocks of 4 output rows (2 pairs): pair cp -> PSUM bank (il) partitions
   64cp+32h+co = row y0+2cp+h. 2 banks per block, bufs=4 -> all 8 banks.
 - Drains are full [128, 510] bias-adds psum->bf16, alternating ScalarE
   (img0) / VectorE (img1). Row block 127 is y0=506 (recomputes rows
   506/507) so all drains stay uniform; host takes rows 508/509 from it.
"""
import numpy as np
from ml_dtypes import bfloat16

import concourse.bass as bass
import concourse.tile as tile
from concourse import bacc, mybir
from concourse.bass_utils import run_bass_kernel_spmd
from contextlib import ExitStack

F32 = mybir.dt.float32
BF16 = mybir.dt.bfloat16

N_FULL, C, H, W = 16, 32, 512, 512
HO = WO = 510
N_CORES = 8
N_PER = N_FULL // N_CORES          # 2 images per core
N_STRIPS = H // 32                 # 16 strips of 32 input rows
N_BLOCKS = 128                     # 4-output-row blocks (block 127: y0=506)


def _block_y0(mb):
    return 4 * mb if mb < N_BLOCKS - 1 else 506


CHUNK_COLS = 4096                  # per-chunk free dim: 8 t-groups = 16 rows
N_CHUNKS = 32                      # 512 rows / 16 rows per chunk
CHUNK_AHEAD = 6                    # chunks issued ahead of the consuming block
X_BUFS = 8                         # chunk ring depth (8 MB SBUF)
OB_BLOCKS = 8                      # output blocks batched per [128, 8*WO] tile


def _build():
    nc = bacc.Bacc("TRN2", target_bir_lowering=False, debug=False, num_devices=1)
    x_d = nc.dram_tensor("x", [N_STRIPS // 2, 128, 16384], BF16,
                         kind="ExternalInput").ap()
    w_d = nc.dram_tensor("w", [128, 384], BF16, kind="ExternalInput").ap()
    b_d = nc.dram_tensor("b", [128, 1], F32, kind="ExternalInput").ap()
    # rows 0..15: il0 groups (blocks 8p..8p+7); rows 16..32: il1 groups
    # staggered by 4 blocks (g covers blocks 8g-4..8g+3), so 1MB stores fire
    # every 4 blocks instead of 2MB bursts every 8.
    o_d = nc.dram_tensor("out", [33, 128, OB_BLOCKS * WO],
                         BF16, kind="ExternalOutput").ap()

    with tile.TileContext(nc) as tc, ExitStack() as ctx:
        const_pool = ctx.enter_context(tc.tile_pool(name="const", bufs=1))
        x_pool = ctx.enter_context(tc.tile_pool(name="xs", bufs=X_BUFS))
        ps_pool = ctx.enter_context(tc.tile_pool(name="ps", bufs=4, space="PSUM"))
        o_pool = ctx.enter_context(tc.tile_pool(name="ob", bufs=3))

        wv = const_pool.tile([128, 384], BF16)
        nc.sync.dma_start(wv[:], w_d[:])
        bt = const_pool.tile([128, 1], F32)
        nc.sync.dma_start(bt[:], b_d[:])

        xtiles = {}
        obtiles = {}

        # Input is DMAed in 512 KB chunks (4 KB descriptors) so the DMA
        # engines' packet-granular round-robin between the load queue and the
        # store queue stays fair; 4 MB strip-pair loads monopolized the
        # engines for ~10 us per packet, starving stores -> ob recycle ->
        # drains -> PSUM -> PE stalls (15 us + ~4 us per strip boundary).
        def load_chunk(c):
            # Loads issue from sync (SP ring), stores from scalar (ACT ring):
            # separate rings so the engines round-robin the two queues at
            # matched ~2.3us packet granularity (one shared FIFO ring was
            # tried: 216us vs 207us, head-of-line blocking). Loads must NOT
            # issue from scalar: the ~0.6us dma_start sequencer cost lands
            # between bias-drains and delays PSUM recycle -> ~1us PE stalls.
            xa = x_pool.tile([128, CHUNK_COLS], BF16, tag="x", name=f"xc_{c}")
            s2, lc = divmod(c, 16384 // CHUNK_COLS)
            nc.scalar.dma_start(
                xa[:], x_d[s2][:, CHUNK_COLS * lc:CHUNK_COLS * (lc + 1)])
            xtiles[2 * c] = (xa, 0)
            xtiles[2 * c + 1] = (xa, CHUNK_COLS // 2)

        def emit_block(mb):
            y0 = _block_y0(mb)
            banks = {}
            for il in range(N_PER):
                banks[il] = ps_pool.tile([128, 512], F32, tag=f"ps{il}",
                                         name=f"ps{il}_{mb}")
            for step in range(6):
                w, kw = divmod(step, 3)
                for il in range(N_PER):
                    for cp in range(2):
                        rw = y0 + 2 * cp + 2 * w       # window rows rw, rw+1
                        xa, base = xtiles[rw // 8]
                        off = base + 512 * ((rw % 8) // 2) + kw
                        nc.tensor.matmul(
                            banks[il][64 * cp:64 * cp + 64, 0:WO],
                            wv[64 * il:64 * il + 64,
                               64 * (3 * w + kw):64 * (3 * w + kw) + 64],
                            xa[64 * il:64 * il + 64, off:off + WO],
                            start=(step == 0), stop=(step == 5),
                            skip_group_check=True,
                            tile_position=(64 * il, 64 * cp),
                        )
            # OB_BLOCKS consecutive blocks share one [128, OB_BLOCKS*510]
            # output tile per image -> 1MB stores whose per-engine packet
            # duration matches the 1MB input chunks (fair round-robin).
            # il1 groups are staggered +4 blocks; first/last groups store in
            # 0.5MB halves so the post-last-matmul tail is short.
            for il in range(N_PER):
                g, half = divmod(mb + 4 * il, OB_BLOCKS)
                if half == 0 or mb == 0:
                    obtiles[il] = o_pool.tile([128, OB_BLOCKS * WO], BF16,
                                              tag=f"ob{il}", name=f"ob{il}_{g}")
                ob = obtiles[il]
                src = banks[il][0:128, 0:WO]
                dst = ob[:, half * WO:half * WO + WO]
                if il == 0:
                    nc.scalar.activation(
                        dst, src, mybir.ActivationFunctionType.Identity,
                        bias=bt[:])
                else:
                    nc.vector.tensor_scalar_add(dst, src, bt[:])
                row = 16 * il + g
                hw = OB_BLOCKS * WO // 2
                last_g = 15 + il                 # 15 for il0, 16 for il1
                first_half_g = (half == OB_BLOCKS // 2 - 1 and g == last_g)
                if half == OB_BLOCKS - 1:
                    if (il == 1 and g == 0) or g == last_g:
                        nc.sync.dma_start(o_d[row][:, hw:], ob[:, hw:])
                    else:
                        nc.sync.dma_start(o_d[row], ob[:])
                elif first_half_g:
                    nc.sync.dma_start(o_d[row][:, 0:hw], ob[:, 0:hw])

        # chunk 0 loads as two 0.5MB halves so the first matmul's input
        # dependency is half as deep (head trim).
        for i in range(2):
            xh = x_pool.tile([128, CHUNK_COLS // 2], BF16, tag="x0", bufs=2,
                             name=f"xc0_{i}")
            nc.scalar.dma_start(
                xh[:], x_d[0][:, (CHUNK_COLS // 2) * i:(CHUNK_COLS // 2) * (i + 1)])
            xtiles[i] = (xh, 0)

        next_chunk = 1
        for mb in range(N_BLOCKS):
            y0 = _block_y0(mb)
            target = min((y0 + 5) // 16 + CHUNK_AHEAD, N_CHUNKS - 1)
            while next_chunk <= target:
                load_chunk(next_chunk)
                next_chunk += 1
            emit_block(mb)

    nc.compile()
    return nc


def _prep_inputs(x, weight, bias):
    """Host-side shard + relayout. Returns per-core in_maps."""
    x = np.asarray(x, dtype=np.float32)
    weight = np.asarray(weight, dtype=np.float32)
    bias = np.asarray(bias, dtype=np.float32)

    # x[2i+il, ci, 32(2*s2+sodd)+2t+q, w]
    #   -> xs[i, s2, 64*il+32*q+ci, 8192*sodd + 512*t + w]
    xr = x.reshape(N_CORES, N_PER, C, N_STRIPS // 2, 2, 16, 2, W)
    xr = xr.transpose(0, 3, 1, 6, 2, 4, 5, 7)   # core, s2, il, q, ci, sodd, t, w
    xs = np.ascontiguousarray(xr).reshape(N_CORES, N_STRIPS // 2, 128, 16384)
    xs = xs.astype(bfloat16)

    # wv[64il + 32q + ci, 64*(3w+kw) + 32h + co] = weight[co, ci, 2w+q-h, kw]
    # (zero when kh = 2w+q-h is outside [0, 3))
    wk = np.zeros((2, 32, 6, 2, 32), dtype=np.float32)  # q, ci, (w,kw), h, co
    for w in range(2):
        for kw in range(3):
            for q in range(2):
                for h in range(2):
                    kh = 2 * w + q - h
                    if 0 <= kh <= 2:
                        wk[q, :, 3 * w + kw, h, :] = weight[:, :, kh, kw].T
    wv = wk.transpose(0, 1, 2, 3, 4).reshape(64, 384)
    wv = np.tile(wv, (2, 1)).astype(bfloat16)
    bt = np.tile(bias, 4)[:, None].astype(np.float32)

    return [{"x": xs[i], "w": wv, "b": bt} for i in range(N_CORES)]


def _unpack_output(results):
    """results: list of 8 dicts with 'out' [33, 128, 4080] bf16.
    Rows 0..15: il0 groups; rows 16..32: il1 groups staggered +4 blocks."""
    dev = np.stack([r["out"] for r in results], axis=0)

    def asm(part):
        # part [cores, G, 128 part = 64cp+32h+co, 8half*510] ->
        # [cores, C, 32G composed rows = 32g+4half+2cp+h, 510]
        G = part.shape[1]
        p = part.reshape(N_CORES, G, 2, 2, C, OB_BLOCKS, WO)
        p = p.transpose(0, 4, 1, 5, 2, 3, 6)
        return p.reshape(N_CORES, C, 32 * G, WO)

    il0 = asm(dev[:, 0:16])                  # composed row = 4mb + 2cp + h
    il1 = asm(dev[:, 16:33])[:, :, 16:528]   # shift out the -4-block stagger
    full = np.stack([il0, il1], axis=1)      # [cores, il, C, 512, WO]
    out = np.empty((N_FULL, C, HO, WO), dtype=np.float32)
    o = out.reshape(N_CORES, N_PER, C, HO, WO)
    o[:, :, :, :508, :] = full[:, :, :, :508, :]
    # block 127 recomputes at y0=506: composed rows 510/511 = true 508/509
    o[:, :, :, 508:510, :] = full[:, :, :, 510:512, :]
    return out


_NC = None


def kernel(x, weight, bias):
    global _NC
    if _NC is None:
        _NC = _build()
    in_maps = _prep_inputs(x, weight, bias)
    res = run_bass_kernel_spmd(_NC, in_maps, core_ids=list(range(N_CORES)))
    return _unpack_output(res.results)



# revision 28
# speedup vs baseline: 1.0995x; 1.0095x over previous
"""Trainium2 Bass kernel for CustomConv2d:
  x [16, 32, 512, 512] f32, weight [32, 32, 3, 3] f32, bias [32] f32
  -> out [16, 32, 510, 510] f32   (stride 1, VALID padding, + bias)

Data-parallel over batch: 2 images per core across 8 NeuronCores.

v9 design (v8 + fine-grained paced input chunks):
 - Host converts x/weight to bf16 and relayouts x into the exact SBUF strip
   layout. Input is DMAed in 512KB chunks of [128, 2048] (4KB descriptors,
   8 output rows each), issued ~10 chunks ahead of the consuming block, so
   the DMA engines' packet-granular round-robin between the load and store
   queues stays fair. (v8's 4MB pair loads made ~262KB/engine packets that
   starved the store queue 9:1, backing up ob recycle -> drains -> PSUM ->
   15us + periodic ~4us PE stalls; PE is the real floor at ~182us.) Output
   accumulates four blocks per [128, 2040] bf16 tile (4080B descriptors)
   and is unshuffled + upcast to f32 on the host.
 - SBUF x layout: strip s holds input rows 32s..32s+31; partition
   64*img + 32*(r%2) + ci, free offset 512*((r%32)//2) + w. One [128, 8192]
   bf16 tile per strip covers both images of the core.
 - Compute: per output row pair (y, y+1), y even: one 6-MM accumulation
   chain of [K=64, M=64] matmuls -- 3 kw taps x 2 two-row input windows,
   with 3-of-4 weight blocks useful per MM (75%, the ceiling for 2-row
   windows). Chains are tile-pure and banks row-half-pure: HW crashes if an
   accumulation group spans PE row tiles or a PSUM bank is written by more
   than one row quadrant (verified by probing), and the PE sustains only ~8
   concurrent matmul streams, so 4 tiles of [64,64] keep the whole array
   busy with big, cheap-to-issue matmuls (3072 total; LDWEIGHTS has ~90ns
   fixed cost and ~2.6x concurrency, so 18k+ small matmuls are issue-bound).
 - Blocks of 4 output rows (2 pairs): pair cp -> PSUM bank (il) partitions
   64cp+32h+co = row y0+2cp+h. 2 banks per block, bufs=4 -> all 8 banks.
 - Drains are full [128, 510] bias-adds psum->bf16, alternating ScalarE
   (img0) / VectorE (img1). Row block 127 is y0=506 (recomputes rows
   506/507) so all drains stay uniform; host takes rows 508/509 from it.
"""
import numpy as np
from ml_dtypes import bfloat16

import concourse.bass as bass
import concourse.tile as tile
from concourse import bacc, mybir
from concourse.bass_utils import run_bass_kernel_spmd
from contextlib import ExitStack

F32 = mybir.dt.float32
BF16 = mybir.dt.bfloat16

N_FULL, C, H, W = 16, 32, 512, 512
HO = WO = 510
N_CORES = 8
N_PER = N_FULL // N_CORES          # 2 images per core
N_STRIPS = H // 32                 # 16 strips of 32 input rows
N_BLOCKS = 128                     # 4-output-row blocks (block 127: y0=506)


def _block_y0(mb):
    return 4 * mb if mb < N_BLOCKS - 1 else 506


CHUNK_COLS = 8192                  # per-chunk free dim: 16 t-groups = 32 rows
N_CHUNKS = 16                      # 512 rows / 32 rows per chunk
CHUNK_AHEAD = 3                    # chunks issued ahead of the consuming block
X_BUFS = 5                         # chunk ring depth (10.5 MB SBUF)
OB_BLOCKS = 16                     # output blocks batched per [128, 16*WO] tile


def _build():
    nc = bacc.Bacc("TRN2", target_bir_lowering=False, debug=False, num_devices=1)
    x_d = nc.dram_tensor("x", [N_STRIPS // 2, 128, 16384], BF16,
                         kind="ExternalInput").ap()
    w_d = nc.dram_tensor("w", [128, 384], BF16, kind="ExternalInput").ap()
    b_d = nc.dram_tensor("b", [128, 1], F32, kind="ExternalInput").ap()
    # rows 0..7: il0 groups (blocks 16p..16p+15); rows 8..16: il1 groups
    # staggered by 8 blocks (g covers blocks 16g-8..16g+7): 2MB stores and
    # 2MB loads alternate with matched ~4.6us per-engine packets, and the
    # dma_start count (and its ~0.4us/engine sem-completion cost) is halved.
    o_d = nc.dram_tensor("out", [17, 128, OB_BLOCKS * WO],
                         BF16, kind="ExternalOutput").ap()

    with tile.TileContext(nc) as tc, ExitStack() as ctx:
        const_pool = ctx.enter_context(tc.tile_pool(name="const", bufs=1))
        x_pool = ctx.enter_context(tc.tile_pool(name="xs", bufs=X_BUFS))
        ps_pool = ctx.enter_context(tc.tile_pool(name="ps", bufs=4, space="PSUM"))
        o_pool = ctx.enter_context(tc.tile_pool(name="ob", bufs=3))

        wv = const_pool.tile([128, 384], BF16)
        nc.sync.dma_start(wv[:], w_d[:])
        bt = const_pool.tile([128, 1], F32)
        nc.sync.dma_start(bt[:], b_d[:])

        xtiles = {}
        obtiles = {}

        # Input is DMAed in 512 KB chunks (4 KB descriptors) so the DMA
        # engines' packet-granular round-robin between the load queue and the
        # store queue stays fair; 4 MB strip-pair loads monopolized the
        # engines for ~10 us per packet, starving stores -> ob recycle ->
        # drains -> PSUM -> PE stalls (15 us + ~4 us per strip boundary).
        def load_chunk(c):
            # Loads issue from sync (SP ring), stores from scalar (ACT ring):
            # separate rings so the engines round-robin the two queues at
            # matched ~2.3us packet granularity (one shared FIFO ring was
            # tried: 216us vs 207us, head-of-line blocking). Loads must NOT
            # issue from scalar: the ~0.6us dma_start sequencer cost lands
            # between bias-drains and delays PSUM recycle -> ~1us PE stalls.
            xa = x_pool.tile([128, CHUNK_COLS], BF16, tag="x", name=f"xc_{c}")
            s2, lc = divmod(c, 16384 // CHUNK_COLS)
            nc.scalar.dma_start(
                xa[:], x_d[s2][:, CHUNK_COLS * lc:CHUNK_COLS * (lc + 1)])
            for k in range(4):
                xtiles[4 * c + k] = (xa, 2048 * k)

        def emit_block(mb):
            y0 = _block_y0(mb)
            banks = {}
            for il in range(N_PER):
                banks[il] = ps_pool.tile([128, 512], F32, tag=f"ps{il}",
                                         name=f"ps{il}_{mb}")
            for step in range(6):
                w, kw = divmod(step, 3)
                for il in range(N_PER):
                    for cp in range(2):
                        rw = y0 + 2 * cp + 2 * w       # window rows rw, rw+1
                        xa, base = xtiles[rw // 8]
                        off = base + 512 * ((rw % 8) // 2) + kw
                        nc.tensor.matmul(
                            banks[il][64 * cp:64 * cp + 64, 0:WO],
                            wv[64 * il:64 * il + 64,
                               64 * (3 * w + kw):64 * (3 * w + kw) + 64],
                            xa[64 * il:64 * il + 64, off:off + WO],
                            start=(step == 0), stop=(step == 5),
                            skip_group_check=True,
                            tile_position=(64 * il, 64 * cp),
                        )
            # OB_BLOCKS consecutive blocks share one [128, OB_BLOCKS*510]
            # output tile per image; stores alternate with loads at matched
            # packet sizes. il1 groups staggered +8 blocks; edge groups store
    # in pieces so the post-last-matmul tail is ~2x0.5MB.
            for il in range(N_PER):
                g, half = divmod(mb + 8 * il, OB_BLOCKS)
                if half == 0 or mb == 0:
                    obtiles[il] = o_pool.tile([128, OB_BLOCKS * WO], BF16,
                                              tag=f"ob{il}", name=f"ob{il}_{g}")
                ob = obtiles[il]
                src = banks[il][0:128, 0:WO]
                dst = ob[:, half * WO:half * WO + WO]
                if il == 0:
                    nc.scalar.activation(
                        dst, src, mybir.ActivationFunctionType.Identity,
                        bias=bt[:])
                else:
                    nc.vector.tensor_scalar_add(dst, src, bt[:])
                row = 8 * il + g
                pieces = []                      # (half_lo, half_hi) to store
                if il == 0:
                    if g < 7 and half == 15:
                        pieces = [(0, 16)]
                    elif g == 7 and half in (7, 11, 13, 15):
                        pieces = [{7: (0, 8), 11: (8, 12),
                                   13: (12, 14), 15: (14, 16)}[half]]
                else:
                    if g == 0 and half == 15:
                        pieces = [(8, 16)]
                    elif 1 <= g <= 7 and half == 15:
                        pieces = [(0, 16)]
                    elif g == 8 and half in (3, 5, 7):
                        pieces = [{3: (0, 4), 5: (4, 6), 7: (6, 8)}[half]]
                for lo, hi in pieces:
                    nc.sync.dma_start(o_d[row][:, lo * WO:hi * WO],
                                      ob[:, lo * WO:hi * WO])

        # chunk 0 loads as four 0.5MB pieces so the first matmul's input
        # dependency is shallow (head trim).
        for i in range(4):
            xh = x_pool.tile([128, 2048], BF16, tag="x0", bufs=4,
                             name=f"xc0_{i}")
            nc.scalar.dma_start(
                xh[:], x_d[0][:, 2048 * i:2048 * (i + 1)])
            xtiles[i] = (xh, 0)

        next_chunk = 1
        for mb in range(N_BLOCKS):
            y0 = _block_y0(mb)
            target = min((y0 + 5) // 32 + CHUNK_AHEAD, N_CHUNKS - 1)
            while next_chunk <= target:
                load_chunk(next_chunk)
                next_chunk += 1
            emit_block(mb)

    nc.compile()
    return nc


def _prep_inputs(x, weight, bias):
    """Host-side shard + relayout. Returns per-core in_maps."""
    x = np.asarray(x, dtype=np.float32)
    weight = np.asarray(weight, dtype=np.float32)
    bias = np.asarray(bias, dtype=np.float32)

    # x[2i+il, ci, 32(2*s2+sodd)+2t+q, w]
    #   -> xs[i, s2, 64*il+32*q+ci, 8192*sodd + 512*t + w]
    xr = x.reshape(N_CORES, N_PER, C, N_STRIPS // 2, 2, 16, 2, W)
    xr = xr.transpose(0, 3, 1, 6, 2, 4, 5, 7)   # core, s2, il, q, ci, sodd, t, w
    xs = np.ascontiguousarray(xr).reshape(N_CORES, N_STRIPS // 2, 128, 16384)
    xs = xs.astype(bfloat16)

    # wv[64il + 32q + ci, 64*(3w+kw) + 32h + co] = weight[co, ci, 2w+q-h, kw]
    # (zero when kh = 2w+q-h is outside [0, 3))
    wk = np.zeros((2, 32, 6, 2, 32), dtype=np.float32)  # q, ci, (w,kw), h, co
    for w in range(2):
        for kw in range(3):
            for q in range(2):
                for h in range(2):
                    kh = 2 * w + q - h
                    if 0 <= kh <= 2:
                        wk[q, :, 3 * w + kw, h, :] = weight[:, :, kh, kw].T
    wv = wk.transpose(0, 1, 2, 3, 4).reshape(64, 384)
    wv = np.tile(wv, (2, 1)).astype(bfloat16)
    bt = np.tile(bias, 4)[:, None].astype(np.float32)

    return [{"x": xs[i], "w": wv, "b": bt} for i in range(N_CORES)]


def _unpack_output(results):
    """results: list of 8 dicts with 'out' [17, 128, 8160] bf16.
    Rows 0..7: il0 groups; rows 8..16: il1 groups staggered +8 blocks."""
    dev = np.stack([r["out"] for r in results], axis=0)

    def asm(part):
        # part [cores, G, 128 part = 64cp+32h+co, 16half*510] ->
        # [cores, C, 64G composed rows = 64g+4half+2cp+h, 510]
        G = part.shape[1]
        p = part.reshape(N_CORES, G, 2, 2, C, OB_BLOCKS, WO)
        p = p.transpose(0, 4, 1, 5, 2, 3, 6)
        return p.reshape(N_CORES, C, 64 * G, WO)

    il0 = asm(dev[:, 0:8])                   # composed row = 4mb + 2cp + h
    il1 = asm(dev[:, 8:17])[:, :, 32:544]    # shift out the -8-block stagger
    full = np.stack([il0, il1], axis=1)      # [cores, il, C, 512, WO]
    out = np.empty((N_FULL, C, HO, WO), dtype=np.float32)
    o = out.reshape(N_CORES, N_PER, C, HO, WO)
    o[:, :, :, :508, :] = full[:, :, :, :508, :]
    # block 127 recomputes at y0=506: composed rows 510/511 = true 508/509
    o[:, :, :, 508:510, :] = full[:, :, :, 510:512, :]
    return out


_NC = None


def kernel(x, weight, bias):
    global _NC
    if _NC is None:
        _NC = _build()
    in_maps = _prep_inputs(x, weight, bias)
    res = run_bass_kernel_spmd(_NC, in_maps, core_ids=list(range(N_CORES)))
    return _unpack_output(res.results)

